# revision 84
# baseline (speedup 1.0000x reference)
"""Trainium2 Bass kernel for nn_CapXLayer (CapsNet-style layer).

Sharding: data-parallel over batch. 8 batches -> 8 NeuronCores, one batch
per core. All parameters replicated. Full inputs in, full output out.

Per-core dataflow (one batch, CH-layout [channels, pixels], px chunks of 512,
processed stage-major in groups of 4 chunks so every engine queue always has
independent cross-chunk work):

  conv:    per-band relu(x) -> conv1 (1x1 grouped) -> conv2 (3x3 grouped
           SAME, 9 full-range taps into one PSUM using a 65-wide row layout
           whose shared zero column serves as both left and right pad)
           -> u_pt[th] tiles (j-layout, bias folded at the PSUM->SBUF copy)
  layouts: q = 16*oc + od            (s tiles; ALSO the output channel order,
                                      so the tail needs no permutation and
                                      output DMAs are contiguous)
           j = 32*(oc>>1) + 16*(oc&1) + 4*icg + odw   (u_pt rows, th=(t,h),
                                      ic = 4h+icg, od = 4t+odw)
           r = 16*oc + ic            ("scattered" rows for per-(ic,oc) values:
                                      nsq/g/c/b/d; rows 16oc+8.. are unused
                                      garbage kept finite by zero mask columns)
           replications are quadrant-local under these layouts: c -> cb[h]
           via DVE stream_shuffle, s16 -> drep[t] via PE mask-matmul +
           ACT/DVE psum copies (CFG knobs, tuned on the cost model).
  routing: squash factor g = n/((0.5+n)*sqrt(n+1e-6)) via ACT Sqrt + DVE
           STT/reciprocal_approx; iterations split into a sqrt-table half
           and a sigmoid-table half, issued stage-major across 4 chunks so
           engines always hold independent cross-chunk work and act-table
           switches are amortized.
  tail:    spatial capsule attention in natural layout: global mean folded
           into the od-reduce mask, per-chunk mean/var partials, one packed
           DRAM broadcast roundtrip, then sig-mult + residual + store.
"""

import numpy as np

import concourse.bass as bass
import concourse.bacc as bacc
import concourse.tile as tile
import concourse.mybir as mybir
from concourse.bass_utils import run_bass_kernel_spmd

F32 = mybir.dt.float32
F32R = mybir.dt.float32r
BF16 = mybir.dt.bfloat16
AF = mybir.ActivationFunctionType
OP = mybir.AluOpType

IC, IND, MID, OC, OD = 8, 16, 32, 8, 16
B, H, W = 8, 64, 64
PX = H * W            # 4096
CS = 512              # pixels per chunk
NCH = PX // CS        # 8 chunks
G = 4                 # chunks in flight (stage-major group)

# tap (0,0) first: it covers the full output range, so it carries start=True
# and zeroes the psum; the clipped edge taps accumulate subranges after it
TAPS = [(0, 0)] + [(dy, dx) for dy in (-1, 0, 1) for dx in (-1, 0, 1)
                   if (dy, dx) != (0, 0)]

# j-layout helpers (u_pt rows): j = 32*(oc>>1) + 16*(oc&1) + 4*icg + odw
_j = np.arange(128)
J_OC = 2 * (_j >> 5) + ((_j >> 4) & 1)
J_ICG = (_j >> 2) & 3
J_ODW = _j & 3
# q-layout (s rows): q = 16*oc + od
_q = np.arange(128)
Q_OC = _q >> 4
Q_OD = _q & 15

# stream_shuffle masks (32-entry, per-quadrant; quadrant = oc>>1 everywhere);
# cb masks indexed by chunk-pair parity p (scattered rows 16oc + 8p + ic)
_i = np.arange(32)
_i_oc1 = _i >> 4
_i_icg = (_i >> 2) & 3
_i_odw = _i & 3
MASK_CB = [[list(16 * _i_oc1 + 8 * p + 4 * h + _i_icg) for h in range(2)]
           for p in range(2)]
MASK_DREP = [list(16 * _i_oc1 + 4 * t + _i_odw) for t in range(4)]


# ---------------------------------------------------------------- host prep
def _prep_consts(w1, b1, w2, b2, w3, b3, attn_w, attn_b):
    """Precompute matmul-ready weight layouts and constant matrices."""
    import ml_dtypes
    c = {}
    # conv1 lhsT: [128, 128]; rows 64h..64h+63 hold half h's lhsT so the
    # lhsT slice shares its base partition with the rhs x-slice
    w1L = np.zeros((128, 128), np.float32)
    for h in range(2):
        for g in range(4):
            gg = h * 4 + g
            w1L[64 * h + g * 16:64 * h + (g + 1) * 16,
                g * 32:(g + 1) * 32] = w1[gg * 32:(gg + 1) * 32, :, 0, 0].T
    c["w1L"] = w1L
    # conv2 lhsT: [128, 2, 9, 128]
    w2L = np.zeros((128, 2, 9, 128), np.float32)
    for h in range(2):
        for t, (dy, dx) in enumerate(TAPS):
            for g in range(4):
                gg = h * 4 + g
                w2L[g * 32:(g + 1) * 32, h, t, g * 32:(g + 1) * 32] = \
                    w2[gg * 32:(gg + 1) * 32, :, dy + 1, dx + 1].T
    c["w2L"] = w2L
    # biases as per-partition columns
    c["b1s"] = np.stack([b1[0:128], b1[128:256]], axis=1).astype(np.float32)
    c["b2s"] = np.stack([b2[0:128], b2[128:256]], axis=1).astype(np.float32)
    # conv3 lhsT (u_pt production): w3P[k, th, j], k = 32*icg + mid
    w3P = np.zeros((128, 8, 128), np.float32)
    b3P = np.zeros((128, 8), np.float32)
    for th in range(8):
        t, h = th >> 1, th & 1
        ch_full = (4 * h + J_ICG) * 128 + 16 * J_OC + 4 * t + J_ODW
        for j in range(128):
            k = J_ICG[j] * 32 + np.arange(MID)
            w3P[k, th, j] = w3[ch_full[j], :, 0, 0]
        b3P[:, th] = b3[ch_full]
    c["w3P"] = w3P
    c["b3P"] = b3P
    # accum masks: product rows j -> s rows q = 16*oc + 4t + odw, summing
    # (icg, h) via the 8-matmul PSUM accumulation. accMh folds iter-1's 0.5.
    accM = np.zeros((128, 4, 128), np.float32)
    for t in range(4):
        accM[_j, t, 16 * J_OC + 4 * t + J_ODW] = 1.0
    c["accM"] = accM.astype(ml_dtypes.bfloat16)
    c["accMh"] = (0.5 * accM).astype(ml_dtypes.bfloat16)
    # reduce masks: product rows j -> scattered rows r = 16*oc + 8*p + ic
    # (parity p packs a chunk PAIR into one tile), summing (t, odw) via th
    # accumulation
    redD = np.zeros((128, 2, 8, 128), np.float32)
    for p in range(2):
        for th in range(8):
            h = th & 1
            redD[_j, p, th, 16 * J_OC + 8 * p + 4 * h + J_ICG] = 1.0
    c["redD"] = redD.astype(ml_dtypes.bfloat16)
    # ns reduce: s rows q -> scattered rows 16*oc + 8*p + ic, replicated
    # over ic
    onesB = np.zeros((128, 2, 128), np.float32)
    for p in range(2):
        for ic in range(IC):
            onesB[_q, p, 16 * Q_OC + 8 * p + ic] = 1.0
    c["onesB"] = onesB.astype(ml_dtypes.bfloat16)
    # attention tail constants (avg packed [64,512], partition = 8c+oc)
    onesA = np.zeros((128, 8), np.float32)
    onesA[_q, Q_OC] = 1.0
    c["onesA"] = onesA
    # selrep[:, c, :]: [64, 8, 128] -- replicate rows 8c..8c+7 (the chunk's
    # [8,CS] sigmoid block) over od into q rows
    selrep = np.zeros((64, 8, 128), np.float32)
    for cc in range(NCH):
        selrep[cc * 8 + Q_OC, cc, _q] = 1.0
    c["selrep"] = selrep.astype(ml_dtypes.bfloat16)
    c["aw_c"] = attn_w.reshape(OC, 1).astype(np.float32).copy()
    ab64 = np.broadcast_to(attn_b.reshape(1, OC), (NCH, OC))
    c["ab64"] = ab64.reshape(64, 1).astype(np.float32).copy()
    c["zpad"] = np.zeros((128, 66), np.float32)
    _prep_rep_masks(c)
    return c


F32_CONSTS = {"b1s", "b2s", "b3P", "aw_c", "ab64"}
BF16_CONSTS = {"accM", "accMh", "redD", "onesB", "selrep", "drepM",
               "cbM", "onesA"}

CONST_SHAPES = {
    "w1L": [128, 128], "w2L": [128, 2, 9, 128], "w3P": [128, 8, 128],
    "b1s": [128, 2], "b2s": [128, 2], "b3P": [128, 8],
    "accM": [128, 4, 128], "accMh": [128, 4, 128],
    "redD": [128, 2, 8, 128], "onesB": [128, 2, 128],
    "onesA": [128, 8],
    "selrep": [64, 8, 128], "aw_c": [8, 1], "ab64": [64, 1],
    "zpad": [128, 66],
    "drepM": [128, 4, 128], "cbM": [128, 2, 128],
}

# h1 band layout: 10 rows of 65 (64 data cols + a shared zero pad column at
# 65r), so the dx=+-1 taps read the pad instead of wrapping to the next row.
H1W = 65
H1SZ = H1W * 11


# engine-assignment knobs (tuned via TimelineSim sweep)
CFG = {
    "rr_mod": 6,        # every rr_mod'th elementwise TT goes to Pool
    "drep_mode": "pe",   # "dve" shuffle | "pe" mask-matmul + copies
    "cb_mode": "dve",    # "dve" shuffle | "pe" mask-matmul + copies
    "upt_dve": 0,       # of every 4 u_pt copies, this many go to DVE
}


def _prep_rep_masks(c):
    """PE-matmul replication masks for drep/cb (used when the *_mode knobs
    select "pe"): lhsT [128 src rows, 128 j rows] bf16."""
    import ml_dtypes
    drepM = np.zeros((128, 4, 128), np.float32)
    for t in range(4):
        drepM[16 * J_OC + 4 * t + J_ODW, t, _j] = 1.0
    c["drepM"] = drepM.astype(ml_dtypes.bfloat16)
    cbM = np.zeros((128, 2, 128), np.float32)
    for h in range(2):
        cbM[16 * J_OC + 4 * h + J_ICG, h, _j] = 1.0
    c["cbM"] = cbM.astype(ml_dtypes.bfloat16)


def build_nc(num_devices=8, stage=99):
    nc = bacc.Bacc("TRN2", target_bir_lowering=False, debug=False,
                   num_devices=num_devices)

    io = {}
    io["x"] = nc.dram_tensor("x", [128, PX], F32R, kind="ExternalInput").ap()
    for name, shp in CONST_SHAPES.items():
        dt = (F32 if name in F32_CONSTS else
              BF16 if name in BF16_CONSTS else F32R)
        io[name] = nc.dram_tensor(name, shp, dt, kind="ExternalInput").ap()
    out_dram = nc.dram_tensor("out", [128, PX], F32, kind="ExternalOutput").ap()

    with tile.TileContext(nc) as tc:
        _body(tc, io, out_dram, stage)
    nc.compile()
    return nc


def _body(tc, io, out_dram, stage=99):
    nc = tc.nc

    import contextlib
    ctx = contextlib.ExitStack()
    with ctx:
        consts = ctx.enter_context(tc.tile_pool(name="consts", bufs=1))
        persist = ctx.enter_context(tc.tile_pool(name="persist", bufs=1))
        x_sb = persist.tile([128, PX], F32R, name="x_sb", tag="x_sb")
        cs_t = {}

        def load_const(name):
            shp = CONST_SHAPES[name]
            dt = (F32 if name in F32_CONSTS else
                  BF16 if name in BF16_CONSTS else F32R)
            t = consts.tile(shp, dt, name=name, tag=name)
            nc.sync.dma_start(out=t[:], in_=io[name])
            cs_t[name] = t

        for name in CONST_SHAPES:
            load_const(name)
        # split the x load so the first conv band starts after ~1/8 of it
        for cc in range(NCH):
            xsl = slice(cc * CS, (cc + 1) * CS)
            nc.sync.dma_start(out=x_sb[:, xsl], in_=io["x"][:, xsl])
        sf_sb = persist.tile([128, PX], BF16, name="sf", tag="sf")
        avg64 = persist.tile([64, CS], F32, name="avg64", tag="avg64")
        cb_eps = persist.tile([128, 1], F32, name="cb_eps", tag="cb_eps")
        nc.vector.memset(cb_eps[:], 1e-6)
        zp = cs_t["zpad"]
        # persistent double-buffered h1 band tiles; pad stripes zeroed once
        h1t = {}
        for h in range(2):
            for slot in range(2):
                t = persist.tile([128, H1SZ], F32R, name=f"h1_{h}_{slot}",
                                 tag=f"h1_{h}_{slot}")
                nc.sync.dma_start(
                    out=t[:, 0:11 * H1W].rearrange(
                        "p (a b) -> p a b", b=H1W)[:, :, 0:1],
                    in_=zp[:, 0:11].rearrange("p (a b) -> p a b", b=1))
                h1t[(h, slot)] = t

        # ------------------------------------------------ pools
        ph2ps = contextlib.ExitStack()
        hb = ph2ps.enter_context(tc.tile_pool(name="hb", bufs=2))
        h2p = ph2ps.enter_context(tc.tile_pool(name="h2p", bufs=G))
        upp = ph2ps.enter_context(tc.tile_pool(name="upp", bufs=1))
        sm = ph2ps.enter_context(tc.tile_pool(name="sm", bufs=1))
        smt = ph2ps.enter_context(tc.tile_pool(name="smt", bufs=2))
        scr = ph2ps.enter_context(tc.tile_pool(name="scr", bufs=2))
        pcv = ph2ps.enter_context(
            tc.tile_pool(name="pcv", bufs=2, space="PSUM"))
        pred = ph2ps.enter_context(
            tc.tile_pool(name="pred", bufs=2, space="PSUM"))
        pd = ph2ps.enter_context(
            tc.tile_pool(name="pd", bufs=2, space="PSUM"))
        psa = ph2ps.enter_context(
            tc.tile_pool(name="psa", bufs=2, space="PSUM"))

        # TT engine round-robin: most to DVE, every rr_mod'th to Pool
        rr = {"i": 0, "c": 0}

        def tt_eng():
            rr["i"] += 1
            return nc.gpsimd if rr["i"] % CFG["rr_mod"] == 0 else nc.vector

        def rep_tiles(src, mask3, shuf_masks, idxs, mode, tagp):
            """Replicated [128, CS] bf16 tiles of src rows, either by DVE
            stream_shuffle or by PE mask-matmul + ACT psum copies."""
            out = []
            for k in idxs:
                r_t = scr.tile([128, CS], BF16, name=f"{tagp}{k}",
                               tag=f"{tagp}{k}")
                if mode == "dve":
                    nc.vector.stream_shuffle(out=r_t[:], in_=src[:],
                                             mask=shuf_masks[k])
                else:
                    ps = pcv.tile([128, CS], F32, name="reps", tag="cvps")
                    nc.tensor.matmul(ps[:], mask3[:, k, :], src[:],
                                     start=True, stop=True)
                    nc.scalar.copy(out=r_t[:], in_=ps[:])
                out.append(r_t)
            return out

        # ------------------------------------------------ conv stages
        def conv1_band(c):
            r_lo = max(8 * c - 1, 0)
            r_hi = min(8 * c + 9, H)
            n = r_hi - r_lo
            idx_lo = r_lo - (8 * c - 1)
            rxb = hb.tile([128, 10 * W], F32R, name="rxb", tag="rxb")
            nc.scalar.activation(out=rxb[:, 0:n * W],
                                 in_=x_sb[:, r_lo * W:r_hi * W], func=AF.Relu)
            h1b = [h1t[(h, c % 2)] for h in range(2)]
            for h in range(2):
                # image-boundary rows are zeroed (the slot holds stale data
                # from chunk c-2 there)
                if c == 0:
                    nc.sync.dma_start(out=h1b[h][:, 1:H1W], in_=zp[:, 0:64])
                if c == NCH - 1:
                    nc.sync.dma_start(out=h1b[h][:, 9 * H1W + 1:10 * H1W],
                                      in_=zp[:, 0:64])
                view = h1b[h][:, 0:10 * H1W].rearrange(
                    "p (a b) -> p a b", b=H1W)
                k1 = n // 2
                for ro, k in ((0, k1), (k1, n - k1)):
                    ps = pcv.tile([128, CS], F32, name="cvps", tag="cvps")
                    nc.tensor.matmul(
                        ps[:, 0:k * W],
                        cs_t["w1L"][h * 64:(h + 1) * 64, :],
                        rxb[h * 64:(h + 1) * 64, ro * W:(ro + k) * W],
                        start=True, stop=True)
                    nc.scalar.activation(
                        out=view[:, idx_lo + ro:idx_lo + ro + k, 1:H1W],
                        in_=ps[:, 0:k * W].rearrange("p (a b) -> p a b", a=k),
                        func=AF.Relu, bias=cs_t["b1s"][:, h:h + 1], scale=1.0)
            return h1b

        def conv2_band(c, h1b):
            h2b = [h2p.tile([128, CS], F32R, name=f"h2b{h}", tag=f"h2b{h}")
                   for h in range(2)]
            for h in range(2):
                ps = pcv.tile([128, CS], F32, name="cvps", tag="cvps")
                for t, (dy, dx) in enumerate(TAPS):
                    s = (1 + dy) * H1W + 1 + dx
                    rhs = h1b[h][:, s:s + 8 * H1W].rearrange(
                        "p (a b) -> p a b", b=H1W)[:, :, 0:W]
                    nc.tensor.matmul(
                        ps[:],
                        cs_t["w2L"][:, h, t, :],
                        rhs,
                        start=(t == 0), stop=(t == len(TAPS) - 1))
                nc.scalar.activation(
                    out=h2b[h][:], in_=ps[:],
                    func=AF.Relu, bias=cs_t["b2s"][:, h:h + 1], scale=1.0)
            return h2b

        def conv3_upt(sl, h2b):
            """u_pt[th] = w3P[:,th,:]^T @ h2b[h] + b3P (bias folded at the
            PSUM->SBUF copy). Relu'd conv2 output in, j-layout bf16 out."""
            u_pt = []
            for th in range(8):
                ps = pcv.tile([128, CS], F32, name="cvps", tag="cvps")
                nc.tensor.matmul(ps[:], cs_t["w3P"][:, th, :],
                                 h2b[th & 1][:], start=True, stop=True)
                u_t = upp.tile([128, CS], BF16, name=f"u{th}",
                               tag=f"u{sl}_{th}")
                if th % 4 < CFG["upt_dve"]:
                    nc.vector.tensor_scalar(
                        out=u_t[:], in0=ps[:],
                        scalar1=cs_t["b3P"][:, th:th + 1], scalar2=None,
                        op0=OP.add)
                else:
                    nc.scalar.activation(out=u_t[:], in_=ps[:],
                                         func=AF.Identity,
                                         bias=cs_t["b3P"][:, th:th + 1],
                                         scale=1.0)
                u_pt.append(u_t)
            return u_pt

        # ------------------------------------------------ routing pieces
        def g_chain(n_sb, gpool, gtag, pool_mult=False):
            """g = n / ((0.5+n) * sqrt(n+1e-6)) -> bf16 [128, CS].
            ACT Sqrt (stage-batched to amortize table loads), Pool STT,
            DVE reciprocal."""
            rt = smt.tile([128, CS], F32, name="g_rt", tag="g_rt")
            nc.scalar.activation(out=rt[:], in_=n_sb[:], func=AF.Sqrt,
                                 bias=cb_eps[:], scale=1.0)
            den = smt.tile([128, CS], F32, name="g_den", tag="g_den")
            nc.vector.scalar_tensor_tensor(out=den[:], in0=n_sb[:], scalar=0.5,
                                           in1=rt[:], op0=OP.add, op1=OP.mult)
            rg = smt.tile([128, CS], F32, name="g_rg", tag="g_rg")
            nc.vector.reciprocal_approx_fast(out=rg[:], in_=den[:])
            g_t = gpool.tile([128, CS], BF16, name="g_g", tag=gtag)
            eng = nc.gpsimd if pool_mult else nc.vector
            eng.tensor_tensor(out=g_t[:], in0=n_sb[:], in1=rg[:],
                              op=OP.mult)
            return g_t

        def accum_pass(u_pt, cT, pi, masks, s_ps):
            """s_ps[q] = sum_(icg,h) cb*u_pt; cb[h] = replicated rows of the
            pair-packed scattered c tile (parity pi selects the chunk)."""
            cb = rep_tiles(cT, cs_t["cbM"], MASK_CB[pi], range(2),
                           CFG["cb_mode"], "cb")
            for th in range(8):
                t, h = th >> 1, th & 1
                p_t = scr.tile([128, CS], BF16, name="pp", tag="pp")
                tt_eng().tensor_tensor(out=p_t[:], in0=u_pt[th][:],
                                       in1=cb[h][:], op=OP.mult)
                nc.tensor.matmul(s_ps[:], masks[:, t, :], p_t[:],
                                 start=(th == 0), stop=(th == 7))

        def d_mults(u_pt, s16, pi, red_ps, first, last):
            """red_ps[16oc+8pi+ic] += sum_od u_pt*srep for one chunk of the
            pair; drep[t] = replicated rows of s16 (q-natural)."""
            drep = rep_tiles(s16, cs_t["drepM"], MASK_DREP, range(4),
                             CFG["drep_mode"], "dr")
            for th in range(8):
                t = th >> 1
                q_t = scr.tile([128, CS], BF16, name="qq", tag="qq")
                tt_eng().tensor_tensor(out=q_t[:], in0=u_pt[th][:],
                                       in1=drep[t][:], op=OP.mult)
                nc.tensor.matmul(red_ps[:], cs_t["redD"][:, pi, th, :], q_t[:],
                                 start=(first and th == 0),
                                 stop=(last and th == 7))

        # ------------------------------------------------ chunk/pair state
        # per-chunk slots hold u tiles and s16; per-(ic,oc) quantities pack a
        # chunk PAIR into one [128, CS] tile (parity = c & 1), halving the
        # squash-chain / sigmoid / b-update work
        st = [dict() for _ in range(G)]
        pst = [dict() for _ in range(2)]

        def s_nsq(pp, c0):
            nsq_ps = pred.tile([128, CS], F32, name="red", tag="red")
            for pi, cc in enumerate((c0, c0 + 1)):
                u_pt = st[cc % G]["u"]
                for th in range(8):
                    sq_t = scr.tile([128, CS], BF16, name="sq", tag="sq")
                    tt_eng().tensor_tensor(out=sq_t[:], in0=u_pt[th][:],
                                           in1=u_pt[th][:], op=OP.mult)
                    nc.tensor.matmul(nsq_ps[:], cs_t["redD"][:, pi, th, :],
                                     sq_t[:], start=(pi == 0 and th == 0),
                                     stop=(pi == 1 and th == 7))
            n_sb = smt.tile([128, CS], F32, name="nsq", tag="nsq")
            nc.scalar.copy(out=n_sb[:], in_=nsq_ps[:])
            pst[pp]["g_u"] = g_chain(n_sb, sm, f"gu_{pp}")

        def s_iter_a(pp, c0, it):
            """Sqrt-table half of a routing iteration for one chunk pair:
            accum -> s16 -> ns -> g -> d -> b update. No Sigmoid here so the
            act table is stable across the whole stage."""
            g_u = pst[pp]["g_u"]
            if it == 1:
                cT, masks = g_u, cs_t["accMh"]
            else:
                cT, masks = pst[pp]["ct2"], cs_t["accM"]
            ns_ps = pred.tile([128, CS], F32, name="red", tag="red")
            for pi, cc in enumerate((c0, c0 + 1)):
                sl = cc % G
                u_pt = st[sl]["u"]
                s_ps = psa.tile([128, CS], F32, name="sacc", tag="sacc")
                accum_pass(u_pt, cT, pi, masks, s_ps)
                s16 = sm.tile([128, CS], BF16, name="s16", tag=f"s16_{sl}")
                nc.scalar.copy(out=s16[:], in_=s_ps[:])
                st[sl]["s16"] = s16
                # squash factor of s
                ssq = scr.tile([128, CS], BF16, name="ssq", tag="ssq")
                nc.scalar.activation(out=ssq[:], in_=s16[:], func=AF.Square)
                nc.tensor.matmul(ns_ps[:], cs_t["onesB"][:, pi, :], ssq[:],
                                 start=(pi == 0), stop=(pi == 1))
            nsb = smt.tile([128, CS], F32, name="nsb", tag="nsb")
            nc.scalar.copy(out=nsb[:], in_=ns_ps[:])
            g_i = g_chain(nsb, smt, "g_i")
            # d = sum_od u*s ; b += d*g_u*g_i
            d_ps = pd.tile([128, CS], F32, name="dred", tag="dred")
            for pi, cc in enumerate((c0, c0 + 1)):
                sl = cc % G
                d_mults(st[sl]["u"], st[sl]["s16"], pi, d_ps,
                        first=(pi == 0), last=(pi == 1))
            gg = smt.tile([128, CS], BF16, name="gg", tag="gg")
            nc.vector.tensor_tensor(out=gg[:], in0=g_i[:], in1=g_u[:],
                                    op=OP.mult)
            if it == 1:
                b2 = sm.tile([128, CS], F32, name="b2", tag=f"b2_{pp}")
                nc.vector.tensor_tensor(out=b2[:], in0=d_ps[:], in1=gg[:],
                                        op=OP.mult)
                pst[pp]["b2"] = b2
            else:
                tb = smt.tile([128, CS], F32, name="tb", tag="tb")
                nc.vector.tensor_tensor(out=tb[:], in0=d_ps[:], in1=gg[:],
                                        op=OP.mult)
                b3 = sm.tile([128, CS], F32, name="b3", tag=f"b3_{pp}")
                nc.vector.tensor_tensor(out=b3[:], in0=tb[:],
                                        in1=pst[pp]["b2"][:], op=OP.add)
                pst[pp]["b3"] = b3

        def s_iter_b(pp, it):
            """Sigmoid-table half: c = sigmoid(b) (+ct2 for iter 1)."""
            g_u = pst[pp]["g_u"]
            if it == 1:
                c2 = smt.tile([128, CS], BF16, name="c2", tag="c2")
                nc.scalar.activation(out=c2[:], in_=pst[pp]["b2"][:],
                                     func=AF.Sigmoid)
                ct2 = sm.tile([128, CS], BF16, name="ct2", tag=f"ct2_{pp}")
                nc.vector.tensor_tensor(out=ct2[:], in0=c2[:], in1=g_u[:],
                                        op=OP.mult)
                pst[pp]["ct2"] = ct2
            else:
                c3 = sm.tile([128, CS], BF16, name="c3", tag=f"c3_{pp}")
                nc.scalar.activation(out=c3[:], in_=pst[pp]["b3"][:],
                                     func=AF.Sigmoid)
                pst[pp]["c3"] = c3

        mh_parts = persist.tile([128, NCH], F32, name="mh_parts",
                                tag="mh_parts")

        def s_final(c, sl):
            csl = slice(c * CS, (c + 1) * CS)
            sf_ps = psa.tile([128, CS], F32, name="sacc", tag="sacc")
            accum_pass(st[sl]["u"], pst[(c >> 1) & 1]["c3"], c & 1,
                       cs_t["accM"], sf_ps)
            nc.scalar.copy(out=sf_sb[:, csl], in_=sf_ps[:])
            # partial spatial sum for the attention tail's global mean
            nc.vector.reduce_sum(out=mh_parts[:, c:c + 1], in_=sf_sb[:, csl],
                                 axis=mybir.AxisListType.X)

        # ------------------------------------------------ main loop
        for si in range(NCH // G):
            cs = list(range(si * G, (si + 1) * G))
            # chunk-major conv (+ pair nsq/g_u) so routing work is ready as
            # soon as the first pair's convs drain; conv1 runs one chunk
            # ahead so PE never waits on the ACT relu copies of the same chunk
            h1b_cur = {cs[0]: conv1_band(cs[0])}
            for k, c in enumerate(cs):
                if k + 1 < G:
                    h1b_cur[cs[k + 1]] = conv1_band(cs[k + 1])
                h2b = conv2_band(c, h1b_cur.pop(c))
                st[c % G]["u"] = conv3_upt(c % G, h2b)
                if c & 1:
                    s_nsq((c >> 1) & 1, c - 1)
            for it in (1, 2):
                for pp in range(2):
                    s_iter_a(pp, cs[2 * pp], it)
                for pp in range(2):
                    s_iter_b(pp, it)
            for c in cs:
                s_final(c, c % G)

        if stage <= 4:
            ph2ps.close()
            nc.sync.dma_start(out=out_dram, in_=sf_sb[:])
            return

        # ---------------- tail: spatial capsule attention ----------------
        ph2ps.close()
        tailp = ctx.enter_context(tc.tile_pool(name="tailp", bufs=2))
        tt = ctx.enter_context(tc.tile_pool(name="tt", bufs=1))
        dramp = ctx.enter_context(tc.tile_pool(name="dramp", bufs=1,
                                               space="DRAM"))
        ppt = ctx.enter_context(tc.tile_pool(name="ppt", bufs=2, space="PSUM"))

        mh = tt.tile([128, 1], F32, name="mh", tag="mh")
        nc.vector.reduce_sum(out=mh[:], in_=mh_parts[:],
                             axis=mybir.AxisListType.X)
        nc.scalar.mul(mh[:], mh[:], 1.0 / PX)
        # fold the global mean into the od-reduce mask: avg = (mh*onesA)^T sf
        onesAm = tt.tile([128, 8], BF16, name="onesAm", tag="onesAm")
        nc.vector.tensor_scalar(out=onesAm[:], in0=cs_t["onesA"][:],
                                scalar1=mh[:], scalar2=None, op0=OP.mult)

        # avg packed [64, CS] (partition 8c+oc) + per-chunk stat partials
        rs8 = tt.tile([8, NCH], F32, name="rs8", tag="rs8")
        sq8 = tt.tile([8, NCH], F32, name="sq8", tag="sq8")
        for c in range(NCH):
            csl = slice(c * CS, (c + 1) * CS)
            av_ps = ppt.tile([8, CS], F32, name="avgc", tag="avgc")
            nc.tensor.matmul(av_ps[:], onesAm[:], sf_sb[:, csl],
                             start=True, stop=True)
            # compute engines need 32-aligned start partitions; bounce via
            # SBUF and let DMA scatter to partition 8c
            avst = tailp.tile([8, CS], F32, name="avst", tag="avst")
            nc.scalar.copy(out=avst[:], in_=av_ps[:])
            nc.sync.dma_start(out=avg64[8 * c:8 * c + 8, :], in_=avst[:])
            nc.vector.reduce_sum(out=rs8[:, c:c + 1], in_=avst[:],
                                 axis=mybir.AxisListType.X)
            avsq = tailp.tile([8, CS], F32, name="avsq", tag="avsq")
            eng = nc.gpsimd if c % 2 == 0 else nc.vector
            eng.tensor_tensor(out=avsq[:], in0=avst[:], in1=avst[:],
                              op=OP.mult)
            nc.vector.reduce_sum(out=sq8[:, c:c + 1], in_=avsq[:],
                                 axis=mybir.AxisListType.X)

        # stats on partitions 0..7: m = sum/PX, var = (ssq - PX m^2)/(PX-1)
        m8 = tt.tile([8, 1], F32, name="m8", tag="m8")
        nc.vector.reduce_sum(out=m8[:], in_=rs8[:], axis=mybir.AxisListType.X)
        nc.scalar.mul(m8[:], m8[:], 1.0 / PX)
        ss8 = tt.tile([8, 1], F32, name="ss8", tag="ss8")
        nc.vector.reduce_sum(out=ss8[:], in_=sq8[:], axis=mybir.AxisListType.X)
        m2 = tt.tile([8, 1], F32, name="m2", tag="m2")
        nc.vector.tensor_tensor(out=m2[:], in0=m8[:], in1=m8[:], op=OP.mult)
        var8 = tt.tile([8, 1], F32, name="var8", tag="var8")
        nc.vector.scalar_tensor_tensor(out=var8[:], in0=m2[:],
                                       scalar=float(-PX), in1=ss8[:],
                                       op0=OP.mult, op1=OP.add)
        sd8 = tt.tile([8, 1], F32, name="sd8", tag="sd8")
        nc.scalar.activation(out=sd8[:], in_=var8[:], func=AF.Sqrt,
                             bias=0.0, scale=1.0 / (PX - 1))
        nc.scalar.activation(out=sd8[:], in_=sd8[:], func=AF.Identity,
                             bias=cb_eps[:8], scale=1.0)
        rsd8 = tt.tile([8, 1], F32, name="rsd8", tag="rsd8")
        nc.vector.reciprocal(out=rsd8[:], in_=sd8[:])
        # pack (m, rsd*attn_w) and broadcast to the 64 chunk-packed rows in
        # one DRAM roundtrip
        pk = tt.tile([8, 2], F32, name="pk", tag="pk")
        nc.vector.tensor_scalar(out=pk[:, 0:1], in0=m8[:], scalar1=1.0,
                                scalar2=None, op0=OP.mult)
        nc.vector.tensor_tensor(out=pk[:, 1:2], in0=rsd8[:],
                                in1=cs_t["aw_c"][:], op=OP.mult)
        pk_d = dramp.tile([8, 2], F32, name="pk_d", tag="pk_d")
        nc.sync.dma_start(out=pk_d[:], in_=pk[:])
        mrw = tt.tile([64, 2], F32, name="mrw", tag="mrw")
        nc.sync.dma_start(
            out=mrw[:],
            in_=bass.AP(tensor=pk_d.tensor, offset=pk_d.offset,
                        ap=[[0, 8], [2, 8], [1, 2]]))
        cen = tt.tile([64, CS], F32, name="cen", tag="cen")
        nc.vector.tensor_scalar(out=cen[:], in0=avg64[:],
                                scalar1=mrw[:, 0:1], scalar2=None,
                                op0=OP.subtract)
        t2 = tt.tile([64, CS], F32, name="t2", tag="t2")
        nc.vector.tensor_scalar(out=t2[:], in0=cen[:], scalar1=mrw[:, 1:2],
                                scalar2=cs_t["ab64"][:], op0=OP.mult,
                                op1=OP.add)
        sig = tt.tile([64, CS], BF16, name="sig", tag="sig")
        nc.scalar.activation(out=sig[:], in_=t2[:], func=AF.Sigmoid)

        for c in range(NCH):
            csl = slice(c * CS, (c + 1) * CS)
            srep = ppt.tile([128, CS], F32, name="srep", tag="srep")
            nc.tensor.matmul(srep[:], cs_t["selrep"][:, c, :],
                             sig[:], start=True, stop=True)
            # bounce srep to bf16 SBUF (ACT is idle here) so the o1 multiply
            # runs in the DVE 2x mode
            srep16 = tailp.tile([128, CS], BF16, name="srep16", tag="srep16")
            nc.scalar.copy(out=srep16[:], in_=srep[:])
            o1 = tailp.tile([128, CS], BF16, name="o1", tag="o1")
            nc.vector.tensor_tensor(out=o1[:], in0=srep16[:],
                                    in1=sf_sb[:, csl], op=OP.mult)
            o2 = tailp.tile([128, CS], F32, name="o2", tag="o2")
            eng = nc.gpsimd if c % 2 == 0 else nc.vector
            eng.tensor_tensor(out=o2[:], in0=o1[:], in1=x_sb[:, csl],
                              op=OP.add)
            nc.sync.dma_start(out=out_dram[:, c * CS:(c + 1) * CS], in_=o2[:])


# ---------------------------------------------------------------- dispatch
_NC_CACHE = {}


def _get_nc():
    if "nc" not in _NC_CACHE:
        _NC_CACHE["nc"] = build_nc()
    return _NC_CACHE["nc"]


def kernel(x, w1, b1, w2, b2, w3, b3, attn_w, attn_b):
    x = np.ascontiguousarray(np.asarray(x, dtype=np.float32))
    consts = _prep_consts(
        np.asarray(w1, np.float32), np.asarray(b1, np.float32),
        np.asarray(w2, np.float32), np.asarray(b2, np.float32),
        np.asarray(w3, np.float32), np.asarray(b3, np.float32),
        np.asarray(attn_w, np.float32), np.asarray(attn_b, np.float32))
    consts = {k: np.ascontiguousarray(v) for k, v in consts.items()}

    nc = _get_nc()
    in_maps = []
    for b in range(B):
        m = {"x": x[b].reshape(128, PX).copy()}
        m.update(consts)
        in_maps.append(m)
    res = run_bass_kernel_spmd(nc, in_maps, core_ids=list(range(B)))
    out = np.zeros((B, 128, H, W), np.float32)
    for b in range(B):
        out[b] = res.results[b]["out"].reshape(128, H, W)
    return out


# revision 85
# speedup vs baseline: 1.0008x; 1.0008x over previous
"""Trainium2 Bass kernel for nn_CapXLayer (CapsNet-style layer).

Sharding: data-parallel over batch. 8 batches -> 8 NeuronCores, one batch
per core. All parameters replicated. Full inputs in, full output out.

Per-core dataflow (one batch, CH-layout [channels, pixels], px chunks of 512,
processed stage-major in groups of 4 chunks so every engine queue always has
independent cross-chunk work):

  conv:    per-band relu(x) -> conv1 (1x1 grouped) -> conv2 (3x3 grouped
           SAME, 9 full-range taps into one PSUM using a 65-wide row layout
           whose shared zero column serves as both left and right pad)
           -> u_pt[th] tiles (j-layout, bias folded at the PSUM->SBUF copy)
  layouts: q = 16*oc + od            (s tiles; ALSO the output channel order,
                                      so the tail needs no permutation and
                                      output DMAs are contiguous)
           j = 32*(oc>>1) + 16*(oc&1) + 4*icg + odw   (u_pt rows, th=(t,h),
                                      ic = 4h+icg, od = 4t+odw)
           r = 16*oc + ic            ("scattered" rows for per-(ic,oc) values:
                                      nsq/g/c/b/d; rows 16oc+8.. are unused
                                      garbage kept finite by zero mask columns)
           replications are quadrant-local under these layouts: c -> cb[h]
           via DVE stream_shuffle, s16 -> drep[t] via PE mask-matmul +
           ACT/DVE psum copies (CFG knobs, tuned on the cost model).
  routing: squash factor g = n/((0.5+n)*sqrt(n+1e-6)) via ACT Sqrt + DVE
           STT/reciprocal_approx; iterations split into a sqrt-table half
           and a sigmoid-table half, issued stage-major across 4 chunks so
           engines always hold independent cross-chunk work and act-table
           switches are amortized.
  tail:    spatial capsule attention in natural layout: global mean folded
           into the od-reduce mask, per-chunk mean/var partials, one packed
           DRAM broadcast roundtrip, then sig-mult + residual + store.
"""

import numpy as np

import concourse.bass as bass
import concourse.bacc as bacc
import concourse.tile as tile
import concourse.mybir as mybir
from concourse.bass_utils import run_bass_kernel_spmd

F32 = mybir.dt.float32
F32R = mybir.dt.float32r
BF16 = mybir.dt.bfloat16
AF = mybir.ActivationFunctionType
OP = mybir.AluOpType

IC, IND, MID, OC, OD = 8, 16, 32, 8, 16
B, H, W = 8, 64, 64
PX = H * W            # 4096
CS = 512              # pixels per chunk
NCH = PX // CS        # 8 chunks
G = 4                 # chunks in flight (stage-major group)

# tap (0,0) first: it covers the full output range, so it carries start=True
# and zeroes the psum; the clipped edge taps accumulate subranges after it
TAPS = [(0, 0)] + [(dy, dx) for dy in (-1, 0, 1) for dx in (-1, 0, 1)
                   if (dy, dx) != (0, 0)]

# j-layout helpers (u_pt rows): j = 32*(oc>>1) + 16*(oc&1) + 4*icg + odw
_j = np.arange(128)
J_OC = 2 * (_j >> 5) + ((_j >> 4) & 1)
J_ICG = (_j >> 2) & 3
J_ODW = _j & 3
# q-layout (s rows): q = 16*oc + od
_q = np.arange(128)
Q_OC = _q >> 4
Q_OD = _q & 15

# stream_shuffle masks (32-entry, per-quadrant; quadrant = oc>>1 everywhere);
# cb masks indexed by chunk-pair parity p (scattered rows 16oc + 8p + ic)
_i = np.arange(32)
_i_oc1 = _i >> 4
_i_icg = (_i >> 2) & 3
_i_odw = _i & 3
MASK_CB = [[list(16 * _i_oc1 + 8 * p + 4 * h + _i_icg) for h in range(2)]
           for p in range(2)]
MASK_DREP = [list(16 * _i_oc1 + 4 * t + _i_odw) for t in range(4)]


# ---------------------------------------------------------------- host prep
def _prep_consts(w1, b1, w2, b2, w3, b3, attn_w, attn_b):
    """Precompute matmul-ready weight layouts and constant matrices."""
    import ml_dtypes
    c = {}
    # conv1 lhsT: [128, 128]; rows 64h..64h+63 hold half h's lhsT so the
    # lhsT slice shares its base partition with the rhs x-slice
    w1L = np.zeros((128, 128), np.float32)
    for h in range(2):
        for g in range(4):
            gg = h * 4 + g
            w1L[64 * h + g * 16:64 * h + (g + 1) * 16,
                g * 32:(g + 1) * 32] = w1[gg * 32:(gg + 1) * 32, :, 0, 0].T
    c["w1L"] = w1L
    # conv2 lhsT: [128, 2, 9, 128]
    w2L = np.zeros((128, 2, 9, 128), np.float32)
    for h in range(2):
        for t, (dy, dx) in enumerate(TAPS):
            for g in range(4):
                gg = h * 4 + g
                w2L[g * 32:(g + 1) * 32, h, t, g * 32:(g + 1) * 32] = \
                    w2[gg * 32:(gg + 1) * 32, :, dy + 1, dx + 1].T
    c["w2L"] = w2L
    # biases as per-partition columns
    c["b1s"] = np.stack([b1[0:128], b1[128:256]], axis=1).astype(np.float32)
    c["b2s"] = np.stack([b2[0:128], b2[128:256]], axis=1).astype(np.float32)
    # conv3 lhsT (u_pt production): w3P[k, th, j], k = 32*icg + mid
    w3P = np.zeros((128, 8, 128), np.float32)
    b3P = np.zeros((128, 8), np.float32)
    for th in range(8):
        t, h = th >> 1, th & 1
        ch_full = (4 * h + J_ICG) * 128 + 16 * J_OC + 4 * t + J_ODW
        for j in range(128):
            k = J_ICG[j] * 32 + np.arange(MID)
            w3P[k, th, j] = w3[ch_full[j], :, 0, 0]
        b3P[:, th] = b3[ch_full]
    c["w3P"] = w3P
    c["b3P"] = b3P
    # accum masks: product rows j -> s rows q = 16*oc + 4t + odw, summing
    # (icg, h) via the 8-matmul PSUM accumulation. accMh folds iter-1's 0.5.
    accM = np.zeros((128, 4, 128), np.float32)
    for t in range(4):
        accM[_j, t, 16 * J_OC + 4 * t + J_ODW] = 1.0
    c["accM"] = accM.astype(ml_dtypes.bfloat16)
    c["accMh"] = (0.5 * accM).astype(ml_dtypes.bfloat16)
    # reduce masks: product rows j -> scattered rows r = 16*oc + 8*p + ic
    # (parity p packs a chunk PAIR into one tile), summing (t, odw) via th
    # accumulation
    redD = np.zeros((128, 2, 8, 128), np.float32)
    for p in range(2):
        for th in range(8):
            h = th & 1
            redD[_j, p, th, 16 * J_OC + 8 * p + 4 * h + J_ICG] = 1.0
    c["redD"] = redD.astype(ml_dtypes.bfloat16)
    # ns reduce: s rows q -> scattered rows 16*oc + 8*p + ic, replicated
    # over ic
    onesB = np.zeros((128, 2, 128), np.float32)
    for p in range(2):
        for ic in range(IC):
            onesB[_q, p, 16 * Q_OC + 8 * p + ic] = 1.0
    c["onesB"] = onesB.astype(ml_dtypes.bfloat16)
    # attention tail constants (avg packed [64,512], partition = 8c+oc)
    onesA = np.zeros((128, 8), np.float32)
    onesA[_q, Q_OC] = 1.0
    c["onesA"] = onesA
    # selrep[:, c, :]: [64, 8, 128] -- replicate rows 8c..8c+7 (the chunk's
    # [8,CS] sigmoid block) over od into q rows
    selrep = np.zeros((64, 8, 128), np.float32)
    for cc in range(NCH):
        selrep[cc * 8 + Q_OC, cc, _q] = 1.0
    c["selrep"] = selrep.astype(ml_dtypes.bfloat16)
    c["aw_c"] = attn_w.reshape(OC, 1).astype(np.float32).copy()
    ab64 = np.broadcast_to(attn_b.reshape(1, OC), (NCH, OC))
    c["ab64"] = ab64.reshape(64, 1).astype(np.float32).copy()
    c["zpad"] = np.zeros((128, 66), np.float32)
    _prep_rep_masks(c)
    return c


F32_CONSTS = {"b1s", "b2s", "b3P", "aw_c", "ab64"}
BF16_CONSTS = {"accM", "accMh", "redD", "onesB", "selrep", "drepM", "cbM"}

CONST_SHAPES = {
    "w1L": [128, 128], "w2L": [128, 2, 9, 128], "w3P": [128, 8, 128],
    "b1s": [128, 2], "b2s": [128, 2], "b3P": [128, 8],
    "accM": [128, 4, 128], "accMh": [128, 4, 128],
    "redD": [128, 2, 8, 128], "onesB": [128, 2, 128],
    "onesA": [128, 8],
    "selrep": [64, 8, 128], "aw_c": [8, 1], "ab64": [64, 1],
    "zpad": [128, 66],
    "drepM": [128, 4, 128], "cbM": [128, 2, 128],
}

# h1 band layout: 10 rows of 65 (64 data cols + a shared zero pad column at
# 65r), so the dx=+-1 taps read the pad instead of wrapping to the next row.
H1W = 65
H1SZ = H1W * 11


# engine-assignment knobs (tuned via TimelineSim sweep)
CFG = {
    "rr_mod": 6,        # every rr_mod'th elementwise TT goes to Pool
    "drep_mode": "pe",   # "dve" shuffle | "pe" mask-matmul + copies
    "cb_mode": "dve",    # "dve" shuffle | "pe" mask-matmul + copies
    "upt_dve": 0,       # of every 4 u_pt copies, this many go to DVE
}


def _prep_rep_masks(c):
    """PE-matmul replication masks for drep/cb (used when the *_mode knobs
    select "pe"): lhsT [128 src rows, 128 j rows] bf16."""
    import ml_dtypes
    drepM = np.zeros((128, 4, 128), np.float32)
    for t in range(4):
        drepM[16 * J_OC + 4 * t + J_ODW, t, _j] = 1.0
    c["drepM"] = drepM.astype(ml_dtypes.bfloat16)
    cbM = np.zeros((128, 2, 128), np.float32)
    for h in range(2):
        cbM[16 * J_OC + 4 * h + J_ICG, h, _j] = 1.0
    c["cbM"] = cbM.astype(ml_dtypes.bfloat16)


def build_nc(num_devices=8, stage=99):
    nc = bacc.Bacc("TRN2", target_bir_lowering=False, debug=False,
                   num_devices=num_devices)

    io = {}
    io["x"] = nc.dram_tensor("x", [128, PX], F32R, kind="ExternalInput").ap()
    for name, shp in CONST_SHAPES.items():
        dt = (F32 if name in F32_CONSTS else
              BF16 if name in BF16_CONSTS else F32R)
        io[name] = nc.dram_tensor(name, shp, dt, kind="ExternalInput").ap()
    out_dram = nc.dram_tensor("out", [128, PX], F32, kind="ExternalOutput").ap()

    with tile.TileContext(nc) as tc:
        _body(tc, io, out_dram, stage)
    nc.compile()
    return nc


def _body(tc, io, out_dram, stage=99):
    nc = tc.nc

    import contextlib
    ctx = contextlib.ExitStack()
    with ctx:
        consts = ctx.enter_context(tc.tile_pool(name="consts", bufs=1))
        persist = ctx.enter_context(tc.tile_pool(name="persist", bufs=1))
        x_sb = persist.tile([128, PX], F32R, name="x_sb", tag="x_sb")
        cs_t = {}

        def load_const(name):
            shp = CONST_SHAPES[name]
            dt = (F32 if name in F32_CONSTS else
                  BF16 if name in BF16_CONSTS else F32R)
            t = consts.tile(shp, dt, name=name, tag=name)
            nc.sync.dma_start(out=t[:], in_=io[name])
            cs_t[name] = t

        for name in CONST_SHAPES:
            load_const(name)
        # split the x load so the first conv band starts after ~1/8 of it
        for cc in range(NCH):
            xsl = slice(cc * CS, (cc + 1) * CS)
            nc.sync.dma_start(out=x_sb[:, xsl], in_=io["x"][:, xsl])
        sf_sb = persist.tile([128, PX], F32R, name="sf", tag="sf")
        avg64 = persist.tile([64, CS], F32, name="avg64", tag="avg64")
        cb_eps = persist.tile([128, 1], F32, name="cb_eps", tag="cb_eps")
        nc.vector.memset(cb_eps[:], 1e-6)
        zp = cs_t["zpad"]
        # persistent double-buffered h1 band tiles; pad stripes zeroed once
        h1t = {}
        for h in range(2):
            for slot in range(2):
                t = persist.tile([128, H1SZ], F32R, name=f"h1_{h}_{slot}",
                                 tag=f"h1_{h}_{slot}")
                nc.sync.dma_start(
                    out=t[:, 0:11 * H1W].rearrange(
                        "p (a b) -> p a b", b=H1W)[:, :, 0:1],
                    in_=zp[:, 0:11].rearrange("p (a b) -> p a b", b=1))
                h1t[(h, slot)] = t

        # ------------------------------------------------ pools
        ph2ps = contextlib.ExitStack()
        hb = ph2ps.enter_context(tc.tile_pool(name="hb", bufs=2))
        h2p = ph2ps.enter_context(tc.tile_pool(name="h2p", bufs=G))
        upp = ph2ps.enter_context(tc.tile_pool(name="upp", bufs=1))
        sm = ph2ps.enter_context(tc.tile_pool(name="sm", bufs=1))
        smt = ph2ps.enter_context(tc.tile_pool(name="smt", bufs=2))
        scr = ph2ps.enter_context(tc.tile_pool(name="scr", bufs=2))
        pcv = ph2ps.enter_context(
            tc.tile_pool(name="pcv", bufs=2, space="PSUM"))
        pred = ph2ps.enter_context(
            tc.tile_pool(name="pred", bufs=2, space="PSUM"))
        pd = ph2ps.enter_context(
            tc.tile_pool(name="pd", bufs=2, space="PSUM"))
        psa = ph2ps.enter_context(
            tc.tile_pool(name="psa", bufs=2, space="PSUM"))

        # TT engine round-robin: most to DVE, every rr_mod'th to Pool
        rr = {"i": 0, "c": 0}

        def tt_eng():
            rr["i"] += 1
            return nc.gpsimd if rr["i"] % CFG["rr_mod"] == 0 else nc.vector

        def rep_tiles(src, mask3, shuf_masks, idxs, mode, tagp):
            """Replicated [128, CS] bf16 tiles of src rows, either by DVE
            stream_shuffle or by PE mask-matmul + ACT psum copies."""
            out = []
            for k in idxs:
                r_t = scr.tile([128, CS], BF16, name=f"{tagp}{k}",
                               tag=f"{tagp}{k}")
                if mode == "dve":
                    nc.vector.stream_shuffle(out=r_t[:], in_=src[:],
                                             mask=shuf_masks[k])
                else:
                    ps = pcv.tile([128, CS], F32, name="reps", tag="cvps")
                    nc.tensor.matmul(ps[:], mask3[:, k, :], src[:],
                                     start=True, stop=True)
                    nc.scalar.copy(out=r_t[:], in_=ps[:])
                out.append(r_t)
            return out

        # ------------------------------------------------ conv stages
        def conv1_band(c):
            r_lo = max(8 * c - 1, 0)
            r_hi = min(8 * c + 9, H)
            n = r_hi - r_lo
            idx_lo = r_lo - (8 * c - 1)
            rxb = hb.tile([128, 10 * W], F32R, name="rxb", tag="rxb")
            nc.scalar.activation(out=rxb[:, 0:n * W],
                                 in_=x_sb[:, r_lo * W:r_hi * W], func=AF.Relu)
            h1b = [h1t[(h, c % 2)] for h in range(2)]
            for h in range(2):
                # image-boundary rows are zeroed (the slot holds stale data
                # from chunk c-2 there)
                if c == 0:
                    nc.sync.dma_start(out=h1b[h][:, 1:H1W], in_=zp[:, 0:64])
                if c == NCH - 1:
                    nc.sync.dma_start(out=h1b[h][:, 9 * H1W + 1:10 * H1W],
                                      in_=zp[:, 0:64])
                view = h1b[h][:, 0:10 * H1W].rearrange(
                    "p (a b) -> p a b", b=H1W)
                k1 = n // 2
                for ro, k in ((0, k1), (k1, n - k1)):
                    ps = pcv.tile([128, CS], F32, name="cvps", tag="cvps")
                    nc.tensor.matmul(
                        ps[:, 0:k * W],
                        cs_t["w1L"][h * 64:(h + 1) * 64, :],
                        rxb[h * 64:(h + 1) * 64, ro * W:(ro + k) * W],
                        start=True, stop=True)
                    nc.scalar.activation(
                        out=view[:, idx_lo + ro:idx_lo + ro + k, 1:H1W],
                        in_=ps[:, 0:k * W].rearrange("p (a b) -> p a b", a=k),
                        func=AF.Relu, bias=cs_t["b1s"][:, h:h + 1], scale=1.0)
            return h1b

        def conv2_band(c, h1b):
            h2b = [h2p.tile([128, CS], F32R, name=f"h2b{h}", tag=f"h2b{h}")
                   for h in range(2)]
            for h in range(2):
                ps = pcv.tile([128, CS], F32, name="cvps", tag="cvps")
                for t, (dy, dx) in enumerate(TAPS):
                    s = (1 + dy) * H1W + 1 + dx
                    rhs = h1b[h][:, s:s + 8 * H1W].rearrange(
                        "p (a b) -> p a b", b=H1W)[:, :, 0:W]
                    nc.tensor.matmul(
                        ps[:],
                        cs_t["w2L"][:, h, t, :],
                        rhs,
                        start=(t == 0), stop=(t == len(TAPS) - 1))
                nc.scalar.activation(
                    out=h2b[h][:], in_=ps[:],
                    func=AF.Relu, bias=cs_t["b2s"][:, h:h + 1], scale=1.0)
            return h2b

        def conv3_upt(sl, h2b):
            """u_pt[th] = w3P[:,th,:]^T @ h2b[h] + b3P (bias folded at the
            PSUM->SBUF copy). Relu'd conv2 output in, j-layout bf16 out."""
            u_pt = []
            for th in range(8):
                ps = pcv.tile([128, CS], F32, name="cvps", tag="cvps")
                nc.tensor.matmul(ps[:], cs_t["w3P"][:, th, :],
                                 h2b[th & 1][:], start=True, stop=True)
                u_t = upp.tile([128, CS], BF16, name=f"u{th}",
                               tag=f"u{sl}_{th}")
                if th % 4 < CFG["upt_dve"]:
                    nc.vector.tensor_scalar(
                        out=u_t[:], in0=ps[:],
                        scalar1=cs_t["b3P"][:, th:th + 1], scalar2=None,
                        op0=OP.add)
                else:
                    nc.scalar.activation(out=u_t[:], in_=ps[:],
                                         func=AF.Identity,
                                         bias=cs_t["b3P"][:, th:th + 1],
                                         scale=1.0)
                u_pt.append(u_t)
            return u_pt

        # ------------------------------------------------ routing pieces
        def g_chain(n_sb, gpool, gtag, pool_mult=False):
            """g = n / ((0.5+n) * sqrt(n+1e-6)) -> bf16 [128, CS].
            ACT Sqrt (stage-batched to amortize table loads), Pool STT,
            DVE reciprocal."""
            rt = smt.tile([128, CS], F32, name="g_rt", tag="g_rt")
            nc.scalar.activation(out=rt[:], in_=n_sb[:], func=AF.Sqrt,
                                 bias=cb_eps[:], scale=1.0)
            den = smt.tile([128, CS], F32, name="g_den", tag="g_den")
            nc.vector.scalar_tensor_tensor(out=den[:], in0=n_sb[:], scalar=0.5,
                                           in1=rt[:], op0=OP.add, op1=OP.mult)
            rg = smt.tile([128, CS], F32, name="g_rg", tag="g_rg")
            nc.vector.reciprocal_approx_fast(out=rg[:], in_=den[:])
            g_t = gpool.tile([128, CS], BF16, name="g_g", tag=gtag)
            eng = nc.gpsimd if pool_mult else nc.vector
            eng.tensor_tensor(out=g_t[:], in0=n_sb[:], in1=rg[:],
                              op=OP.mult)
            return g_t

        def accum_pass(u_pt, cT, pi, masks, s_ps):
            """s_ps[q] = sum_(icg,h) cb*u_pt; cb[h] = replicated rows of the
            pair-packed scattered c tile (parity pi selects the chunk)."""
            cb = rep_tiles(cT, cs_t["cbM"], MASK_CB[pi], range(2),
                           CFG["cb_mode"], "cb")
            for th in range(8):
                t, h = th >> 1, th & 1
                p_t = scr.tile([128, CS], BF16, name="pp", tag="pp")
                tt_eng().tensor_tensor(out=p_t[:], in0=u_pt[th][:],
                                       in1=cb[h][:], op=OP.mult)
                nc.tensor.matmul(s_ps[:], masks[:, t, :], p_t[:],
                                 start=(th == 0), stop=(th == 7))

        def d_mults(u_pt, s16, pi, red_ps, first, last):
            """red_ps[16oc+8pi+ic] += sum_od u_pt*srep for one chunk of the
            pair; drep[t] = replicated rows of s16 (q-natural)."""
            drep = rep_tiles(s16, cs_t["drepM"], MASK_DREP, range(4),
                             CFG["drep_mode"], "dr")
            for th in range(8):
                t = th >> 1
                q_t = scr.tile([128, CS], BF16, name="qq", tag="qq")
                tt_eng().tensor_tensor(out=q_t[:], in0=u_pt[th][:],
                                       in1=drep[t][:], op=OP.mult)
                nc.tensor.matmul(red_ps[:], cs_t["redD"][:, pi, th, :], q_t[:],
                                 start=(first and th == 0),
                                 stop=(last and th == 7))

        # ------------------------------------------------ chunk/pair state
        # per-chunk slots hold u tiles and s16; per-(ic,oc) quantities pack a
        # chunk PAIR into one [128, CS] tile (parity = c & 1), halving the
        # squash-chain / sigmoid / b-update work
        st = [dict() for _ in range(G)]
        pst = [dict() for _ in range(2)]

        def s_nsq(pp, c0):
            nsq_ps = pred.tile([128, CS], F32, name="red", tag="red")
            for pi, cc in enumerate((c0, c0 + 1)):
                u_pt = st[cc % G]["u"]
                for th in range(8):
                    sq_t = scr.tile([128, CS], BF16, name="sq", tag="sq")
                    tt_eng().tensor_tensor(out=sq_t[:], in0=u_pt[th][:],
                                           in1=u_pt[th][:], op=OP.mult)
                    nc.tensor.matmul(nsq_ps[:], cs_t["redD"][:, pi, th, :],
                                     sq_t[:], start=(pi == 0 and th == 0),
                                     stop=(pi == 1 and th == 7))
            n_sb = smt.tile([128, CS], F32, name="nsq", tag="nsq")
            nc.scalar.copy(out=n_sb[:], in_=nsq_ps[:])
            pst[pp]["g_u"] = g_chain(n_sb, sm, f"gu_{pp}")

        def s_iter_a(pp, c0, it):
            """Sqrt-table half of a routing iteration for one chunk pair:
            accum -> s16 -> ns -> g -> d -> b update. No Sigmoid here so the
            act table is stable across the whole stage."""
            g_u = pst[pp]["g_u"]
            if it == 1:
                cT, masks = g_u, cs_t["accMh"]
            else:
                cT, masks = pst[pp]["ct2"], cs_t["accM"]
            ns_ps = pred.tile([128, CS], F32, name="red", tag="red")
            for pi, cc in enumerate((c0, c0 + 1)):
                sl = cc % G
                u_pt = st[sl]["u"]
                s_ps = psa.tile([128, CS], F32, name="sacc", tag="sacc")
                accum_pass(u_pt, cT, pi, masks, s_ps)
                s16 = sm.tile([128, CS], BF16, name="s16", tag=f"s16_{sl}")
                nc.scalar.copy(out=s16[:], in_=s_ps[:])
                st[sl]["s16"] = s16
                # squash factor of s
                ssq = scr.tile([128, CS], BF16, name="ssq", tag="ssq")
                nc.scalar.activation(out=ssq[:], in_=s16[:], func=AF.Square)
                nc.tensor.matmul(ns_ps[:], cs_t["onesB"][:, pi, :], ssq[:],
                                 start=(pi == 0), stop=(pi == 1))
            nsb = smt.tile([128, CS], F32, name="nsb", tag="nsb")
            nc.scalar.copy(out=nsb[:], in_=ns_ps[:])
            g_i = g_chain(nsb, smt, "g_i")
            # d = sum_od u*s ; b += d*g_u*g_i
            d_ps = pd.tile([128, CS], F32, name="dred", tag="dred")
            for pi, cc in enumerate((c0, c0 + 1)):
                sl = cc % G
                d_mults(st[sl]["u"], st[sl]["s16"], pi, d_ps,
                        first=(pi == 0), last=(pi == 1))
            gg = smt.tile([128, CS], BF16, name="gg", tag="gg")
            nc.vector.tensor_tensor(out=gg[:], in0=g_i[:], in1=g_u[:],
                                    op=OP.mult)
            if it == 1:
                b2 = sm.tile([128, CS], F32, name="b2", tag=f"b2_{pp}")
                nc.vector.tensor_tensor(out=b2[:], in0=d_ps[:], in1=gg[:],
                                        op=OP.mult)
                pst[pp]["b2"] = b2
            else:
                tb = smt.tile([128, CS], F32, name="tb", tag="tb")
                nc.vector.tensor_tensor(out=tb[:], in0=d_ps[:], in1=gg[:],
                                        op=OP.mult)
                b3 = sm.tile([128, CS], F32, name="b3", tag=f"b3_{pp}")
                nc.vector.tensor_tensor(out=b3[:], in0=tb[:],
                                        in1=pst[pp]["b2"][:], op=OP.add)
                pst[pp]["b3"] = b3

        def s_iter_b(pp, it):
            """Sigmoid-table half: c = sigmoid(b) (+ct2 for iter 1)."""
            g_u = pst[pp]["g_u"]
            if it == 1:
                c2 = smt.tile([128, CS], BF16, name="c2", tag="c2")
                nc.scalar.activation(out=c2[:], in_=pst[pp]["b2"][:],
                                     func=AF.Sigmoid)
                ct2 = sm.tile([128, CS], BF16, name="ct2", tag=f"ct2_{pp}")
                nc.vector.tensor_tensor(out=ct2[:], in0=c2[:], in1=g_u[:],
                                        op=OP.mult)
                pst[pp]["ct2"] = ct2
            else:
                c3 = sm.tile([128, CS], BF16, name="c3", tag=f"c3_{pp}")
                nc.scalar.activation(out=c3[:], in_=pst[pp]["b3"][:],
                                     func=AF.Sigmoid)
                pst[pp]["c3"] = c3

        mh_parts = persist.tile([128, NCH], F32, name="mh_parts",
                                tag="mh_parts")

        def s_final(c, sl):
            csl = slice(c * CS, (c + 1) * CS)
            sf_ps = psa.tile([128, CS], F32, name="sacc", tag="sacc")
            accum_pass(st[sl]["u"], pst[(c >> 1) & 1]["c3"], c & 1,
                       cs_t["accM"], sf_ps)
            nc.scalar.copy(out=sf_sb[:, csl], in_=sf_ps[:])
            # partial spatial sum for the attention tail's global mean
            nc.vector.reduce_sum(out=mh_parts[:, c:c + 1], in_=sf_sb[:, csl],
                                 axis=mybir.AxisListType.X)

        # ------------------------------------------------ main loop
        for si in range(NCH // G):
            cs = list(range(si * G, (si + 1) * G))
            # chunk-major conv (+ pair nsq/g_u) so routing work is ready as
            # soon as the first pair's convs drain; conv1 runs one chunk
            # ahead so PE never waits on the ACT relu copies of the same chunk
            h1b_cur = {cs[0]: conv1_band(cs[0])}
            for k, c in enumerate(cs):
                if k + 1 < G:
                    h1b_cur[cs[k + 1]] = conv1_band(cs[k + 1])
                h2b = conv2_band(c, h1b_cur.pop(c))
                st[c % G]["u"] = conv3_upt(c % G, h2b)
                if c & 1:
                    s_nsq((c >> 1) & 1, c - 1)
            for it in (1, 2):
                for pp in range(2):
                    s_iter_a(pp, cs[2 * pp], it)
                for pp in range(2):
                    s_iter_b(pp, it)
            for c in cs:
                s_final(c, c % G)

        if stage <= 4:
            ph2ps.close()
            nc.sync.dma_start(out=out_dram, in_=sf_sb[:])
            return

        # ---------------- tail: spatial capsule attention ----------------
        ph2ps.close()
        tailp = ctx.enter_context(tc.tile_pool(name="tailp", bufs=2))
        tt = ctx.enter_context(tc.tile_pool(name="tt", bufs=1))
        dramp = ctx.enter_context(tc.tile_pool(name="dramp", bufs=1,
                                               space="DRAM"))
        ppt = ctx.enter_context(tc.tile_pool(name="ppt", bufs=2, space="PSUM"))

        mh = tt.tile([128, 1], F32, name="mh", tag="mh")
        nc.vector.reduce_sum(out=mh[:], in_=mh_parts[:],
                             axis=mybir.AxisListType.X)
        nc.scalar.mul(mh[:], mh[:], 1.0 / PX)
        # fold the global mean into the od-reduce mask: avg = (mh*onesA)^T sf
        onesAm = tt.tile([128, 8], F32R, name="onesAm", tag="onesAm")
        nc.vector.tensor_scalar(out=onesAm[:], in0=cs_t["onesA"][:],
                                scalar1=mh[:], scalar2=None, op0=OP.mult)

        # avg packed [64, CS] (partition 8c+oc) + per-chunk stat partials
        rs8 = tt.tile([8, NCH], F32, name="rs8", tag="rs8")
        sq8 = tt.tile([8, NCH], F32, name="sq8", tag="sq8")
        for c in range(NCH):
            csl = slice(c * CS, (c + 1) * CS)
            av_ps = ppt.tile([8, CS], F32, name="avgc", tag="avgc")
            nc.tensor.matmul(av_ps[:], onesAm[:], sf_sb[:, csl],
                             start=True, stop=True)
            # compute engines need 32-aligned start partitions; bounce via
            # SBUF and let DMA scatter to partition 8c
            avst = tailp.tile([8, CS], F32, name="avst", tag="avst")
            nc.scalar.copy(out=avst[:], in_=av_ps[:])
            nc.sync.dma_start(out=avg64[8 * c:8 * c + 8, :], in_=avst[:])
            nc.vector.reduce_sum(out=rs8[:, c:c + 1], in_=avst[:],
                                 axis=mybir.AxisListType.X)
            avsq = tailp.tile([8, CS], F32, name="avsq", tag="avsq")
            eng = nc.gpsimd if c % 2 == 0 else nc.vector
            eng.tensor_tensor(out=avsq[:], in0=avst[:], in1=avst[:],
                              op=OP.mult)
            nc.vector.reduce_sum(out=sq8[:, c:c + 1], in_=avsq[:],
                                 axis=mybir.AxisListType.X)

        # stats on partitions 0..7: m = sum/PX, var = (ssq - PX m^2)/(PX-1)
        m8 = tt.tile([8, 1], F32, name="m8", tag="m8")
        nc.vector.reduce_sum(out=m8[:], in_=rs8[:], axis=mybir.AxisListType.X)
        nc.scalar.mul(m8[:], m8[:], 1.0 / PX)
        ss8 = tt.tile([8, 1], F32, name="ss8", tag="ss8")
        nc.vector.reduce_sum(out=ss8[:], in_=sq8[:], axis=mybir.AxisListType.X)
        m2 = tt.tile([8, 1], F32, name="m2", tag="m2")
        nc.vector.tensor_tensor(out=m2[:], in0=m8[:], in1=m8[:], op=OP.mult)
        var8 = tt.tile([8, 1], F32, name="var8", tag="var8")
        nc.vector.scalar_tensor_tensor(out=var8[:], in0=m2[:],
                                       scalar=float(-PX), in1=ss8[:],
                                       op0=OP.mult, op1=OP.add)
        sd8 = tt.tile([8, 1], F32, name="sd8", tag="sd8")
        nc.scalar.activation(out=sd8[:], in_=var8[:], func=AF.Sqrt,
                             bias=0.0, scale=1.0 / (PX - 1))
        nc.scalar.activation(out=sd8[:], in_=sd8[:], func=AF.Identity,
                             bias=cb_eps[:8], scale=1.0)
        rsd8 = tt.tile([8, 1], F32, name="rsd8", tag="rsd8")
        nc.vector.reciprocal(out=rsd8[:], in_=sd8[:])
        # pack (m, rsd*attn_w) and broadcast to the 64 chunk-packed rows in
        # one DRAM roundtrip
        pk = tt.tile([8, 2], F32, name="pk", tag="pk")
        nc.vector.tensor_scalar(out=pk[:, 0:1], in0=m8[:], scalar1=1.0,
                                scalar2=None, op0=OP.mult)
        nc.vector.tensor_tensor(out=pk[:, 1:2], in0=rsd8[:],
                                in1=cs_t["aw_c"][:], op=OP.mult)
        pk_d = dramp.tile([8, 2], F32, name="pk_d", tag="pk_d")
        nc.sync.dma_start(out=pk_d[:], in_=pk[:])
        mrw = tt.tile([64, 2], F32, name="mrw", tag="mrw")
        nc.sync.dma_start(
            out=mrw[:],
            in_=bass.AP(tensor=pk_d.tensor, offset=pk_d.offset,
                        ap=[[0, 8], [2, 8], [1, 2]]))
        cen = tt.tile([64, CS], F32, name="cen", tag="cen")
        nc.vector.tensor_scalar(out=cen[:], in0=avg64[:],
                                scalar1=mrw[:, 0:1], scalar2=None,
                                op0=OP.subtract)
        t2 = tt.tile([64, CS], F32, name="t2", tag="t2")
        nc.vector.tensor_scalar(out=t2[:], in0=cen[:], scalar1=mrw[:, 1:2],
                                scalar2=cs_t["ab64"][:], op0=OP.mult,
                                op1=OP.add)
        sig = tt.tile([64, CS], BF16, name="sig", tag="sig")
        nc.scalar.activation(out=sig[:], in_=t2[:], func=AF.Sigmoid)

        for c in range(NCH):
            csl = slice(c * CS, (c + 1) * CS)
            srep = ppt.tile([128, CS], F32, name="srep", tag="srep")
            nc.tensor.matmul(srep[:], cs_t["selrep"][:, c, :],
                             sig[:], start=True, stop=True)
            o1 = tailp.tile([128, CS], F32, name="o1", tag="o1")
            nc.vector.tensor_tensor(out=o1[:], in0=srep[:], in1=sf_sb[:, csl],
                                    op=OP.mult)
            o2 = tailp.tile([128, CS], F32, name="o2", tag="o2")
            eng = nc.gpsimd if c % 2 == 0 else nc.vector
            eng.tensor_tensor(out=o2[:], in0=o1[:], in1=x_sb[:, csl],
                              op=OP.add)
            nc.sync.dma_start(out=out_dram[:, c * CS:(c + 1) * CS], in_=o2[:])


# ---------------------------------------------------------------- dispatch
_NC_CACHE = {}


def _get_nc():
    if "nc" not in _NC_CACHE:
        _NC_CACHE["nc"] = build_nc()
    return _NC_CACHE["nc"]


def kernel(x, w1, b1, w2, b2, w3, b3, attn_w, attn_b):
    x = np.ascontiguousarray(np.asarray(x, dtype=np.float32))
    consts = _prep_consts(
        np.asarray(w1, np.float32), np.asarray(b1, np.float32),
        np.asarray(w2, np.float32), np.asarray(b2, np.float32),
        np.asarray(w3, np.float32), np.asarray(b3, np.float32),
        np.asarray(attn_w, np.float32), np.asarray(attn_b, np.float32))
    consts = {k: np.ascontiguousarray(v) for k, v in consts.items()}

    nc = _get_nc()
    in_maps = []
    for b in range(B):
        m = {"x": x[b].reshape(128, PX).copy()}
        m.update(consts)
        in_maps.append(m)
    res = run_bass_kernel_spmd(nc, in_maps, core_ids=list(range(B)))
    out = np.zeros((B, 128, H, W), np.float32)
    for b in range(B):
        out[b] = res.results[b]["out"].reshape(128, H, W)
    return out
